# revision 2
# baseline (speedup 1.0000x reference)
"""Bass/Trainium2 kernel for nn_HardAndLayer (8 NeuronCores, tensor-parallel).

v5: single custom-DVE instruction for the whole per-core computation.

    res[p,t] = OR_j ( w[p,t,j] & nx[p,t,j] )        t = 0..7, j = 0..255

Hand-patched 3-state uop program (seed -> steady -> step):
  - steady: scan state = OR(state, AND(src0, src1)); SUB_DIM_DONE at each
    256-word page boundary jumps to step.
  - step (1 element): state = OR(zero, AND(src0, src1)) -- the scan restarts
    for the new page while consuming its first element.
  - out writes are gated by write_subdim_last, so exactly one word per page
    is written: the page's OR-reduction. out AP is [128, 8].
Everything else (HWDGE-only input DMA, serial compute start, no final wait)
as v4.
"""

import os
import sys

if "/opt/trn_rl_repo" not in sys.path:
    sys.path.insert(0, "/opt/trn_rl_repo")

import copy

import numpy as np

import concourse.bacc as bacc
import concourse.mybir as mybir
from concourse.bass_utils import run_bass_kernel_spmd

OUT, IN = 8192, 8192
NCORES = 8
P = 128
NW = IN // 32            # 256 u32 words per row
RPC = OUT // NCORES      # 1024 rows per core
NT = RPC // P            # 8 row-tiles per core
F32 = mybir.dt.float32
U32 = mybir.dt.uint32

FINAL_WAIT = os.environ.get("K5_FINAL_WAIT", "0") == "1"

_cached = {}


def _register_seg_or_op():
    """Register SEG_OR_ANT: segmented OR-of-AND with one output per page."""
    import concourse.dve_ops as dve_ops_mod
    from concourse.dve_ops import DveOp, _COMPILE_CACHE
    from concourse.dve_spec import Spec, Bin, Src0, Src1, Zero, lower, scan
    from concourse.dve_uop import (
        AluInp,
        AluOp,
        DveOpSpec,
        ENABLE,
        InpSel,
        Trigger,
        UopConfig,
    )

    name = "SEG_OR_ANT"
    if name in dve_ops_mod._SUB_OPCODE_FOR_NAME:
        return next(o for o in dve_ops_mod.OPS if o.name == name)

    def _ref(in0, in1, s0, s1, imm2):
        a = in0.astype(np.uint32) & in1.astype(np.uint32)
        r = np.bitwise_or.reduce(a.reshape(a.shape[0], NT, NW), axis=-1)
        return r

    spec = Spec(
        body=scan(AluOp.BITWISE_OR, Bin(AluOp.BITWISE_AND, Src0, Src1), init=Zero),
        reference=_ref,
    )
    uops = lower(spec, ver="v3")
    assert len(uops) == 2, f"expected [seed, steady], got {len(uops)} uops"
    seed, steady = uops

    # Find the scan stage in steady: ALU op BITWISE_OR with src0=CURR_ALU_OUT.
    scan_stage = None
    for i, blk in enumerate(steady.datapath_config):
        if blk.op == AluOp.BITWISE_OR and blk.alu_src0 == AluInp.CURR_ALU_OUT:
            scan_stage = i
            break
    assert scan_stage is not None, "scan stage not found in steady uop"

    # The seed state routes the scan init (Zero) into the same stage; find the
    # operand selector it uses so the step state can reuse it as the reset.
    zero_src = seed.datapath_config[scan_stage].alu_src0
    # Sanity: that selector must trace back to an input lane wired to ZERO, or
    # be a const path; accept PREV_DELAY_* lanes wired from InpSel.ZERO.
    if AluInp.PREV_DELAY_0 <= zero_src <= AluInp.PREV_DELAY_6:
        lane = int(zero_src) - int(AluInp.PREV_DELAY_0)
        assert steady.inp[lane + 1] == InpSel.ZERO or seed.inp[lane + 1] == InpSel.ZERO, (
            f"seed scan operand lane {lane} is {steady.inp[lane + 1]}, not ZERO"
        )

    # step: like steady, but the scan combine reads the zero lane instead of
    # the accumulated value -> state restarts with this element's AND result.
    step = copy.deepcopy(steady)
    blk = step.datapath_config[scan_stage]
    blk.alu_src0 = zero_src
    step.repeat_count = 1

    STEP_IDX = 2
    steady.trigger = (Trigger.SRC_TENSOR_DONE, Trigger.SUB_DIM_DONE, Trigger.NONE)
    steady.next_uop = (0, STEP_IDX, 0)
    step.trigger = (Trigger.SRC_TENSOR_DONE, Trigger.SUB_DIM_DONE, Trigger.COUNT)
    step.next_uop = (0, STEP_IDX, 1)

    # Gate output writes to the last element of each page.
    steady.out_last_subdim_enable = ENABLE
    step.out_last_subdim_enable = ENABLE

    uops = [seed, steady, step]
    row = max(dve_ops_mod._SUB_OPCODE_FOR_NAME.values()) + 1
    dve_ops_mod._SUB_OPCODE_FOR_NAME[name] = row

    shas = {}
    for ver in ("v3", "v4"):
        vuops = uops if ver == "v3" else None
        if vuops is None:
            vuops = lower(spec, ver="v4")
            seed4, steady4 = vuops
            scan_stage4 = None
            for i, blk4 in enumerate(steady4.datapath_config):
                if blk4.op == AluOp.BITWISE_OR and blk4.alu_src0 == AluInp.CURR_ALU_OUT:
                    scan_stage4 = i
                    break
            zero_src4 = seed4.datapath_config[scan_stage4].alu_src0
            step4 = copy.deepcopy(steady4)
            step4.datapath_config[scan_stage4].alu_src0 = zero_src4
            step4.repeat_count = 1
            steady4.trigger = (Trigger.SRC_TENSOR_DONE, Trigger.SUB_DIM_DONE, Trigger.NONE)
            steady4.next_uop = (0, STEP_IDX, 0)
            step4.trigger = (Trigger.SRC_TENSOR_DONE, Trigger.SUB_DIM_DONE, Trigger.COUNT)
            step4.next_uop = (0, STEP_IDX, 1)
            steady4.out_last_subdim_enable = ENABLE
            step4.out_last_subdim_enable = ENABLE
            vuops = [seed4, steady4, step4]
        opspec = DveOpSpec(name=name, opcode=row, uops=vuops, rd1_en=True)
        shas[ver] = opspec.sha(ver)
        _COMPILE_CACHE[(name, ver)] = opspec

    op = DveOp(name, spec, subdim=True, uops_sha=shas)
    dve_ops_mod.OPS.append(op)
    dve_ops_mod.CUSTOM_DVE_SPECS[name] = spec
    return op


def _build_module():
    op = _register_seg_or_op()
    nc = bacc.Bacc(
        None,
        enable_partition_id=False,
        enable_asserts=False,
        monotonic_sem_count=0,
    )
    main_bb = nc.m.functions[0].blocks[0]
    snapshot = list(main_bb.instructions)

    wx = nc.dram_tensor("wx", [P, 2 * NT * NW], U32, kind="ExternalInput")
    out = nc.dram_tensor("out", [P, NT], U32, kind="ExternalOutput")

    nxs = nc.alloc_sbuf_tensor("nxs", [P, NT, NW], U32)
    ws = nc.alloc_sbuf_tensor("ws", [P, NT, NW], U32)
    res = nc.alloc_sbuf_tensor("res", [P, NT], U32)

    sem_d = nc.alloc_semaphore("din")
    sem_v = nc.alloc_semaphore("vdone")
    sem_o = nc.alloc_semaphore("odone")

    nc.sync.dma_start(nxs[:], wx[:, : NT * NW]).then_inc(sem_d, 16)
    nc.sync.dma_start(ws[:], wx[:, NT * NW :]).then_inc(sem_d, 16)

    nc.vector.wait_ge(sem_d, 32)
    nc.vector._custom_dve(
        op,
        out=res[:].bitcast(F32),
        in0=ws[:].bitcast(F32),
        in1=nxs[:].bitcast(F32),
    ).then_inc(sem_v, 1)

    nc.sync.wait_ge(sem_v, 1)
    nc.sync.dma_start(out[:], res[:]).then_inc(sem_o, 16)
    if FINAL_WAIT:
        nc.sync.wait_ge(sem_o, 16)

    kill_types = ("InstMemset", "InstDrain", "InstEventSemaphore")
    kill = {id(i) for i in snapshot if type(i).__name__ in kill_types}
    main_bb.instructions = [i for i in main_bb.instructions if id(i) not in kill]

    nc.compile()
    return nc


def _pack_bits(bool2d: np.ndarray) -> np.ndarray:
    u8 = np.packbits(bool2d, axis=-1, bitorder="little")
    return u8.view(np.uint32)


def kernel(weights: np.ndarray, x: np.ndarray, **run_kwargs):
    wbits = _pack_bits(np.asarray(weights) != 0)                # [8192, 256]
    nxbits = _pack_bits((~np.asarray(x, dtype=bool))[None, :])  # [1, 256]
    nx_rep = np.broadcast_to(np.tile(nxbits, (1, NT)), (P, NT * NW))

    in_maps = []
    for c in range(NCORES):
        wr = (
            wbits[c * RPC : (c + 1) * RPC]
            .reshape(NT, P, NW)
            .transpose(1, 0, 2)
            .reshape(P, NT * NW)
        )
        in_maps.append(
            {"wx": np.ascontiguousarray(np.concatenate([nx_rep, wr], axis=1))}
        )

    if "nc" not in _cached:
        _cached["nc"] = _build_module()
    nc = _cached["nc"]

    r = run_bass_kernel_spmd(nc, in_maps, core_ids=list(range(NCORES)), **run_kwargs)

    outs = []
    for c in range(NCORES):
        m = r.results[c]["out"]            # [P, NT] u32 OR-reduced words
        outs.append(m.T.reshape(RPC))      # row t*128+p within core
    bits = np.concatenate(outs)            # [8192]
    result = bits == 0
    if run_kwargs:
        return result, r
    return result
